# revision 38
# baseline (speedup 1.0000x reference)
"""GNN NodeBlock (message passing + 3-layer MLP + LayerNorm) on 8 Trainium2 cores.

Strategy (data parallel over nodes):
  - Shard 50000 nodes across 8 cores (6250 each, padded to 6272 = 49*128).
  - The segment-sum runs on the *VectorEngine*, not the TensorEngine: the host
    sorts each core's nodes by in-degree (ascending), packs every node's
    incoming edge features into fixed per-tile slot counts dp[t] (= max degree
    in that 128-node tile; degree sorting makes the padding ~2%), laid out as
    eft[f, node, slot] so one DVE tensor_reduce(axis=X) per tile produces the
    aggregate already in T-layout [96 feat, 128 nodes]. This removes ~47us of
    one-hot matmul work from the TensorEngine, which is the bottleneck.
  - The MLP runs entirely in T-layout (features on partitions, nodes on the
    free dim) with weights stationary: h^T = W.T @ x^T, so no transposes are
    needed between layers. Node features enter pre-transposed from the host
    (with the same degree-sort node permutation; the host un-permutes y).
  - Layer 3 swaps the operands (activations stationary) to produce y in natural
    layout [128 nodes, 512 feats]; bias b3 is added on the VectorEngine.
    LayerNorm then reduces over the free dim: bn_stats/bn_aggr (VectorE) +
    sqrt (ScalarE) + reciprocal (VectorE), applied via one ScalarE activation
    with per-partition scale/bias.
  - All matmuls are bf16 inputs with fp32 PSUM accumulation (~4e-3 L2 rel err).

Everything is compiled once per (dp-pattern, apply_gamma_beta) and cached.
"""

import numpy as np
import ml_dtypes

P = 128
NODE_DIM = 512
EDGE_DIM = 96
HID = 1024
OUT = 512
N_NODES = 50000
N_EDGES = 800000
NCORES = 8
LN_EPS = 1e-5

NPC = N_NODES // NCORES          # 6250 nodes per core
T_TILES = -(-NPC // P)           # 49 node tiles per core
NPAD = T_TILES * P               # 6272
NSHIFT = NPAD - NPC              # 22 zero-degree pad slots at the front
GMAX = 4                         # node tiles per super-tile (NT = 512 free dim)

BF16 = ml_dtypes.bfloat16

_CACHE: dict = {}


# ----------------------------------------------------------------------------
# Bass program
# ----------------------------------------------------------------------------

def _build_program(dp: tuple, apply_gamma_beta: bool):
    import concourse.bass as bass
    import concourse.bacc as bacc
    import concourse.mybir as mybir
    import concourse.tile as tile

    f32 = mybir.dt.float32
    bf16 = mybir.dt.bfloat16
    Act = mybir.ActivationFunctionType
    Alu = mybir.AluOpType
    Axis = mybir.AxisListType

    KD = NODE_DIM // P           # 4 node-feat k-chunks
    KH = HID // P                # 8 hidden k-chunks
    MH = HID // P                # 8 hidden m-chunks
    KD1 = KD + 1                 # + 1 chunk for the 96 agg features

    assert len(dp) == T_TILES
    off = [0] * T_TILES
    for t in range(1, T_TILES):
        off[t] = off[t - 1] + P * dp[t - 1]
    total_cols = off[-1] + P * dp[-1]
    dpmax = max(dp)

    nc = bacc.Bacc("TRN2", target_bir_lowering=False, debug=False)

    # inputs (per core)
    eft_d = nc.declare_dram_parameter("eft", [EDGE_DIM, total_cols], bf16, isOutput=False)
    nfT_d = nc.declare_dram_parameter("nfT", [NODE_DIM, NPAD], bf16, isOutput=False)
    w1_d = nc.declare_dram_parameter("w1", [P, KD1 * MH * P], bf16, isOutput=False)
    w2_d = nc.declare_dram_parameter("w2", [P, KH * MH * P], bf16, isOutput=False)
    w3_d = nc.declare_dram_parameter("w3", [P, KH * OUT], bf16, isOutput=False)
    # cstB: b1T(MH) | b2T(MH); cstLN: gamma(OUT) | beta(OUT) | b3(OUT) | eps(1)
    cstB_d = nc.declare_dram_parameter("cstB", [P, 2 * MH], f32, isOutput=False)
    cstLN_d = nc.declare_dram_parameter("cstLN", [P, 3 * OUT + 1], f32, isOutput=False)
    # cstb3: bf16 [b3(OUT) | ones(P)] on one partition, for the K=1 matmul
    # that folds the b3 add into the final tile's PSUM accumulation
    cstb3_d = nc.declare_dram_parameter("cstb3", [1, OUT + P], bf16, isOutput=False)
    y_d = nc.declare_dram_parameter("y", [NPAD, OUT], bf16, isOutput=True)

    groups = []
    t0 = 0
    while t0 < T_TILES:
        g = min(GMAX, T_TILES - t0)
        groups.append((t0, g))
        t0 += g

    with tile.TileContext(nc) as tc:
        with (
            tc.tile_pool(name="const", bufs=1) as constp,
            tc.tile_pool(name="eft", bufs=6) as eftp,
            tc.tile_pool(name="agg", bufs=3) as aggp,
            tc.tile_pool(name="nfx", bufs=2) as nfxp,
            tc.tile_pool(name="h1", bufs=2) as h1p,
            tc.tile_pool(name="h2", bufs=2) as h2p,
            tc.tile_pool(name="yo", bufs=3) as yop,
            tc.tile_pool(name="st", bufs=8) as stp,
            tc.tile_pool(name="psM", bufs=4, space="PSUM") as psM,
            tc.tile_pool(name="psY", bufs=3, space="PSUM") as psY,
        ):
            cstB_sb = constp.tile([P, 2 * MH], f32)
            nc.sync.dma_start(out=cstB_sb[:], in_=cstB_d[:, :])
            cstb3_sb = constp.tile([1, OUT + P], bf16)
            nc.sync.dma_start(out=cstb3_sb[:], in_=cstb3_d[:, :])
            w1_sb = constp.tile([P, MH * KD1 * P], bf16)
            w2_sb = constp.tile([P, MH * KH * P], bf16)
            w3_sb = constp.tile([P, KH * OUT], bf16)
            cstLN_sb = constp.tile([P, 3 * OUT + 1], f32)

            nfT_ap = nfT_d[:, :].rearrange("(k p) n -> p k n", p=P)

            def emit_reduce(t, aggT, s):
                """Segment-sum of one 128-node tile into aggT[:, s*P:(s+1)*P].

                eft holds each node's incoming edge features padded to dp[t]
                slots; a single DVE add-reduce over the slot axis yields the
                [96 feat, 128 node] aggregate (fp32 internally, bf16 out)."""
                cols = P * dp[t]
                if cols == 0:
                    nc.vector.memset(aggT[:, s * P:(s + 1) * P], 0.0)
                    return
                eft_t = eftp.tile([EDGE_DIM, P * dpmax], bf16, tag="eft",
                                  name="eft_t")
                nc.sync.dma_start(out=eft_t[:, 0:cols],
                                  in_=eft_d[:, off[t]:off[t] + cols])
                with nc.allow_low_precision(
                        reason="DVE reduce accumulates fp32 internally; bf16 "
                               "output matches the baseline agg precision"):
                    nc.vector.tensor_reduce(
                        out=aggT[:, s * P:(s + 1) * P],
                        in_=eft_t[:, 0:cols].rearrange("p (n d) -> p n d",
                                                       d=dp[t]),
                        axis=Axis.X,
                        op=Alu.add,
                    )

            # group 0's aggregation up front (weight DMAs stream in behind it)
            agg_tiles = {}
            agg_tiles[0] = aggp.tile([EDGE_DIM, GMAX * P], bf16, tag="agg",
                                     name="aggT")
            for s in range(groups[0][1]):
                emit_reduce(groups[0][0] + s, agg_tiles[0], s)

            nfx_tiles = {}

            def emit_nfx(gj):
                tstart_j, g_j = groups[gj]
                nfx_tiles[gj] = nfxp.tile([P, KD, GMAX * P], bf16, tag="nfx",
                                          name="nfx")
                nj0 = tstart_j * P
                nc.sync.dma_start(out=nfx_tiles[gj][:, :, 0:g_j * P],
                                  in_=nfT_ap[:, :, nj0:nj0 + g_j * P])

            emit_nfx(0)

            for gi, (tstart, g) in enumerate(groups):
                nt = g * P  # free-dim width of this super-tile
                n0 = tstart * P
                aggT = agg_tiles.pop(gi)
                nfx = nfx_tiles.pop(gi)
                if gi == 0:
                    # per-m weight slices stream in behind group 0's agg work so
                    # layer 1/2 can begin as soon as their own slice lands
                    for m in range(MH):
                        nc.sync.dma_start(
                            out=w1_sb[:, m * KD1 * P:(m + 1) * KD1 * P],
                            in_=w1_d[:, m * KD1 * P:(m + 1) * KD1 * P])

                # ---- layer 1: h1T[m] = relu(W1.T @ xT + b1), x = [nf; agg] ----
                h1 = h1p.tile([P, KH, GMAX * P], bf16, tag="h1")
                for m in range(MH):
                    ps = psM.tile([P, GMAX * P], f32, tag="psM")
                    for k in range(KD):
                        nc.tensor.matmul(
                            out=ps[:, 0:nt],
                            lhsT=w1_sb[:, (m * KD1 + k) * P:(m * KD1 + k + 1) * P],
                            rhs=nfx[:, k, 0:nt],
                            start=(k == 0),
                            stop=False,
                        )
                    nc.tensor.matmul(
                        out=ps[:, 0:nt],
                        lhsT=w1_sb[0:EDGE_DIM, (m * KD1 + KD) * P:(m * KD1 + KD) * P + P],
                        rhs=aggT[:, 0:nt],
                        start=False,
                        stop=True,
                    )
                    nc.scalar.activation(
                        out=h1[:, m, 0:nt], in_=ps[:, 0:nt], func=Act.Relu,
                        bias=cstB_sb[:, m:m + 1],
                    )
                    if gi == 0:
                        nc.sync.dma_start(
                            out=w2_sb[:, m * KH * P:(m + 1) * KH * P],
                            in_=w2_d[:, m * KH * P:(m + 1) * KH * P])

                # node features of the NEXT group start streaming here so
                # layer 1 of group gi+1 never waits on them
                if gi + 1 < len(groups):
                    emit_nfx(gi + 1)

                # ---- layer 2 ----
                h2 = h2p.tile([P, KH, GMAX * P], bf16, tag="h2")
                for m in range(MH):
                    ps = psM.tile([P, GMAX * P], f32, tag="psM")
                    for k in range(KH):
                        nc.tensor.matmul(
                            out=ps[:, 0:nt],
                            lhsT=w2_sb[:, (m * KH + k) * P:(m * KH + k + 1) * P],
                            rhs=h1[:, k, 0:nt],
                            start=(k == 0),
                            stop=(k == KH - 1),
                        )
                    nc.scalar.activation(
                        out=h2[:, m, 0:nt], in_=ps[:, 0:nt], func=Act.Relu,
                        bias=cstB_sb[:, MH + m:MH + m + 1],
                    )
                    if gi == 0 and m < 2:
                        if m == 0:
                            nc.sync.dma_start(out=w3_sb[:], in_=w3_d[:, :])
                        else:
                            nc.sync.dma_start(out=cstLN_sb[:], in_=cstLN_d[:, :])

                # ---- layer 3 (nodes on partitions) + LayerNorm ----
                # the next group's eft DMAs + DVE reduces interleave here so
                # its aggT is ready before layer 1 of group gi+1 needs it.
                if gi + 1 < len(groups):
                    tstart_nx, g_nx = groups[gi + 1]
                    agg_tiles[gi + 1] = aggp.tile([EDGE_DIM, GMAX * P], bf16,
                                                  tag="agg", name="aggT")
                else:
                    tstart_nx, g_nx = 0, 0
                for s in range(max(g, g_nx)):
                    if s < g_nx:
                        emit_reduce(tstart_nx + s, agg_tiles[gi + 1], s)
                    if s >= g:
                        continue
                    fold_b3 = (gi == len(groups) - 1)
                    ps_y = psY.tile([P, OUT], f32, tag="psY")
                    for k in range(KH):
                        nc.tensor.matmul(
                            out=ps_y[:],
                            lhsT=h2[:, k, s * P:(s + 1) * P],
                            rhs=w3_sb[:, k * OUT:(k + 1) * OUT],
                            start=(k == 0),
                            stop=(k == KH - 1) and not fold_b3,
                        )
                    if fold_b3:
                        # K=1 ones-row matmul adds b3 inside the accumulation,
                        # shortening the final tile's serial LN chain
                        nc.tensor.matmul(
                            out=ps_y[:],
                            lhsT=cstb3_sb[0:1, OUT:OUT + P],
                            rhs=cstb3_sb[0:1, 0:OUT],
                            start=False,
                            stop=True,
                        )
                    else:
                        # + b3 (broadcast rows) on VectorE, off TensorE's path
                        nc.vector.tensor_tensor(
                            out=ps_y[:], in0=ps_y[:],
                            in1=cstLN_sb[:, 2 * OUT:3 * OUT],
                            op=Alu.add,
                        )
                    st6 = stp.tile([P, 6], f32, tag="st6")
                    nc.vector.bn_stats(st6[:], ps_y[:])
                    mv = stp.tile([P, 2], f32, tag="mv")
                    nc.vector.bn_aggr(mv[:], st6[:])
                    std = stp.tile([P, 1], f32, tag="std")
                    nc.scalar.activation(std[:], mv[:, 1:2], Act.Sqrt,
                                         bias=cstLN_sb[:, 3 * OUT:])
                    rstd = stp.tile([P, 1], f32, tag="rstd")
                    nc.vector.reciprocal(rstd[:], std[:])
                    nmr = stp.tile([P, 1], f32, tag="nmr")
                    nc.vector.tensor_scalar(
                        out=nmr[:], in0=mv[:, 0:1], scalar1=rstd[:], scalar2=-1.0,
                        op0=Alu.mult, op1=Alu.mult,
                    )
                    yn = yop.tile([P, OUT], bf16, tag="yn")
                    nc.scalar.activation(
                        out=yn[:], in_=ps_y[:], func=Act.Identity,
                        bias=nmr[:], scale=rstd[:],
                    )
                    if apply_gamma_beta:
                        nc.vector.tensor_tensor(
                            out=yn[:], in0=yn[:],
                            in1=cstLN_sb[:, 0:OUT], op=Alu.mult,
                        )
                        nc.vector.tensor_tensor(
                            out=yn[:], in0=yn[:],
                            in1=cstLN_sb[:, OUT:2 * OUT], op=Alu.add,
                        )
                    r0 = (tstart + s) * P
                    nc.sync.dma_start(out=y_d[r0:r0 + P, :], in_=yn[:])

    nc.compile()
    return nc


# ----------------------------------------------------------------------------
# Host-side sharding / layout prep
# ----------------------------------------------------------------------------

def _prep_core(c, node_feat, edge_feat, dst, order, dp, off, total_cols,
               pos_of_block):
    """Degree-sorted node layout for one core.

    order: ascending-degree permutation of this core's nodes; sorted node i
    sits at padded position NSHIFT + i (tile = pos >> 7, rel = pos & 127).
    eft[f, off[t] + rel*dp[t] + slot] = edge_feat[e, f] for the slot-th
    incoming edge of that node; unused slots stay zero.
    """
    lo = c * NPC
    q = NSHIFT + np.arange(NPC)
    q = pos_of_block[q >> 7] * P + (q & 127)
    pos_of_node = np.empty(NPC, np.int64)
    pos_of_node[order] = q

    sel = np.flatnonzero((dst >= lo) & (dst < lo + NPC))
    p = pos_of_node[dst[sel] - lo]
    eorder = np.argsort(p, kind="stable")
    sel = sel[eorder]
    p = p[eorder]
    # slot index within each node's run
    node_starts = np.zeros(NPAD, np.int64)
    cnts = np.bincount(p, minlength=NPAD)
    np.cumsum(cnts[:-1], out=node_starts[1:])
    slot = np.arange(p.size) - node_starts[p]
    t = p >> 7
    rel = p & 127
    cols = off[t] + rel * dp[t] + slot
    assert slot.max(initial=0) < dp[t].max(initial=1) and cols.max(initial=0) < total_cols

    eft = np.zeros((EDGE_DIM, total_cols), BF16)
    eft[:, cols] = edge_feat[sel].astype(BF16).T

    nfT = np.zeros((NODE_DIM, NPAD), BF16)
    nfT[:, pos_of_node[order]] = node_feat[lo + order].T.astype(BF16)
    return {"eft": eft, "nfT": nfT}


def _prep_shared(W1, b1, W2, b2, W3, b3, gamma, beta):
    KD1 = NODE_DIM // P + 1
    MH = HID // P
    KH = HID // P

    w1p = np.zeros((KD1 * P, HID), np.float32)
    w1p[:NODE_DIM + EDGE_DIM] = W1
    # m-major: col index (m*KD1 + k)*P + j
    w1 = np.ascontiguousarray(
        w1p.reshape(KD1, P, MH, P).transpose(1, 2, 0, 3)).reshape(P, -1).astype(BF16)
    w2 = np.ascontiguousarray(
        W2.reshape(KH, P, MH, P).transpose(1, 2, 0, 3)).reshape(P, -1).astype(BF16)
    w3 = np.ascontiguousarray(
        W3.reshape(KH, P, OUT).transpose(1, 0, 2)).reshape(P, -1).astype(BF16)

    cstB = np.ascontiguousarray(np.concatenate(
        [b1.reshape(MH, P).T, b2.reshape(MH, P).T], axis=1).astype(np.float32))
    cstb3 = np.concatenate(
        [b3.reshape(1, OUT), np.ones((1, P), np.float32)], axis=1).astype(BF16)
    cstLN = np.ascontiguousarray(np.concatenate([
        np.tile(gamma.reshape(1, OUT), (P, 1)),
        np.tile(beta.reshape(1, OUT), (P, 1)),
        np.tile(b3.reshape(1, OUT), (P, 1)),
        np.full((P, 1), LN_EPS, np.float32),
    ], axis=1).astype(np.float32))
    return {"w1": w1, "w2": w2, "w3": w3, "cstB": cstB, "cstb3": cstb3,
            "cstLN": cstLN}


# ----------------------------------------------------------------------------
# Entry point
# ----------------------------------------------------------------------------

def _ensure_axon_hooks_importable():
    """bass_utils imports antenv.axon_hooks when tracing is requested (even via
    the BASS_TRACE env var); provide a no-op stub if the module is absent so
    that path degrades to trace-skipped instead of crashing."""
    try:
        import antenv.axon_hooks  # noqa: F401
    except Exception:
        import sys
        import types
        try:
            import antenv
        except Exception:
            return
        mod = types.ModuleType('antenv.axon_hooks')
        mod._hook = None
        mod.set_axon_ntff_profile_hook = lambda h: setattr(mod, '_hook', h)
        mod.get_axon_ntff_profile_hook = lambda: mod._hook
        sys.modules['antenv.axon_hooks'] = mod
        antenv.axon_hooks = mod


def kernel(node_feat, edge_feat, edge_index, n_nodes, W1, b1, W2, b2, W3, b3,
           gamma, beta, _want_trace=False):
    from concourse.bass_utils import run_bass_kernel_spmd
    _ensure_axon_hooks_importable()

    node_feat = np.asarray(node_feat, dtype=np.float32)
    edge_feat = np.asarray(edge_feat, dtype=np.float32)
    edge_index = np.asarray(edge_index)
    assert int(n_nodes) == N_NODES
    assert node_feat.shape == (N_NODES, NODE_DIM)
    assert edge_feat.shape == (N_EDGES, EDGE_DIM)

    dst = edge_index[1].astype(np.int64)

    # degree-sorted node order per core; global per-tile slot counts dp[t]
    counts = np.bincount(dst, minlength=N_NODES)
    orders = []
    dp = np.zeros(T_TILES, np.int64)
    for c in range(NCORES):
        degc = counts[c * NPC:(c + 1) * NPC]
        order = np.argsort(degc, kind="stable")
        orders.append(order)
        padded = np.zeros(NPAD, np.int64)
        padded[NSHIFT:] = degc[order]
        dp = np.maximum(dp, padded.reshape(T_TILES, P).max(axis=1))
    # tile order: ascending degree, except the final (single-tile) group gets
    # a small tile so its (just-in-time) DVE reduce is short; the fattest
    # tile lands mid-schedule where it has a full group of slack
    seq = np.concatenate([np.arange(4), np.arange(5, T_TILES), [4]])
    pos_of_block = np.empty(T_TILES, np.int64)
    pos_of_block[seq] = np.arange(T_TILES)
    dp = dp[seq]
    off = np.zeros(T_TILES, np.int64)
    np.cumsum(P * dp[:-1], out=off[1:])
    total_cols = int(off[-1] + P * dp[-1])

    gamma = np.asarray(gamma, dtype=np.float32)
    beta = np.asarray(beta, dtype=np.float32)
    apply_gb = not (np.all(gamma == 1.0) and np.all(beta == 0.0))

    key = (tuple(int(x) for x in dp), apply_gb)
    if key not in _CACHE:
        _CACHE[key] = _build_program(key[0], apply_gb)
    nc = _CACHE[key]

    shared = _prep_shared(
        np.asarray(W1, np.float32), np.asarray(b1, np.float32),
        np.asarray(W2, np.float32), np.asarray(b2, np.float32),
        np.asarray(W3, np.float32), np.asarray(b3, np.float32),
        gamma, beta)

    in_maps = []
    for c in range(NCORES):
        m = _prep_core(c, node_feat, edge_feat, dst, orders[c], dp, off,
                       total_cols, pos_of_block)
        m.update(shared)
        in_maps.append(m)

    res = run_bass_kernel_spmd(nc, in_maps, list(range(NCORES)), trace=_want_trace)

    q = NSHIFT + np.arange(NPC)
    q = pos_of_block[q >> 7] * P + (q & 127)
    out = np.empty((N_NODES, OUT), np.float32)
    for c in range(NCORES):
        yc = np.asarray(res.results[c]["y"], dtype=np.float32)
        out[c * NPC + orders[c]] = yc[q]
    if _want_trace:
        kernel.last_results = res
    return out


kernel.last_results = None


# revision 39
# speedup vs baseline: 1.0006x; 1.0006x over previous
"""GNN NodeBlock (message passing + 3-layer MLP + LayerNorm) on 8 Trainium2 cores.

Strategy (data parallel over nodes):
  - Shard 50000 nodes across 8 cores (6250 each, padded to 6272 = 49*128).
  - The segment-sum runs on the *VectorEngine*, not the TensorEngine: the host
    sorts each core's nodes by in-degree (ascending), packs every node's
    incoming edge features into fixed per-tile slot counts dp[t] (= max degree
    in that 128-node tile; degree sorting makes the padding ~2%), laid out as
    eft[f, node, slot] so one DVE tensor_reduce(axis=X) per tile produces the
    aggregate already in T-layout [96 feat, 128 nodes]. This removes ~47us of
    one-hot matmul work from the TensorEngine, which is the bottleneck.
  - The MLP runs entirely in T-layout (features on partitions, nodes on the
    free dim) with weights stationary: h^T = W.T @ x^T, so no transposes are
    needed between layers. Node features enter pre-transposed from the host
    (with the same degree-sort node permutation; the host un-permutes y).
  - Layer 3 swaps the operands (activations stationary) to produce y in natural
    layout [128 nodes, 512 feats]; bias b3 is added on the VectorEngine.
    LayerNorm then reduces over the free dim: bn_stats/bn_aggr (VectorE) +
    sqrt (ScalarE) + reciprocal (VectorE), applied via one ScalarE activation
    with per-partition scale/bias.
  - All matmuls are bf16 inputs with fp32 PSUM accumulation (~4e-3 L2 rel err).

Everything is compiled once per (dp-pattern, apply_gamma_beta) and cached.
"""

import numpy as np
import ml_dtypes

P = 128
NODE_DIM = 512
EDGE_DIM = 96
HID = 1024
OUT = 512
N_NODES = 50000
N_EDGES = 800000
NCORES = 8
LN_EPS = 1e-5

NPC = N_NODES // NCORES          # 6250 nodes per core
T_TILES = -(-NPC // P)           # 49 node tiles per core
NPAD = T_TILES * P               # 6272
NSHIFT = NPAD - NPC              # 22 zero-degree pad slots at the front
GMAX = 4                         # node tiles per super-tile (NT = 512 free dim)

BF16 = ml_dtypes.bfloat16

_CACHE: dict = {}


# ----------------------------------------------------------------------------
# Bass program
# ----------------------------------------------------------------------------

def _build_program(dp: tuple, apply_gamma_beta: bool):
    import concourse.bass as bass
    import concourse.bacc as bacc
    import concourse.mybir as mybir
    import concourse.tile as tile

    f32 = mybir.dt.float32
    bf16 = mybir.dt.bfloat16
    Act = mybir.ActivationFunctionType
    Alu = mybir.AluOpType
    Axis = mybir.AxisListType

    KD = NODE_DIM // P           # 4 node-feat k-chunks
    KH = HID // P                # 8 hidden k-chunks
    MH = HID // P                # 8 hidden m-chunks
    KD1 = KD + 1                 # + 1 chunk for the 96 agg features

    assert len(dp) == T_TILES
    off = [0] * T_TILES
    for t in range(1, T_TILES):
        off[t] = off[t - 1] + P * dp[t - 1]
    total_cols = off[-1] + P * dp[-1]
    dpmax = max(dp)

    nc = bacc.Bacc("TRN2", target_bir_lowering=False, debug=False)

    # inputs (per core)
    eft_d = nc.declare_dram_parameter("eft", [EDGE_DIM, total_cols], bf16, isOutput=False)
    nfT_d = nc.declare_dram_parameter("nfT", [NODE_DIM, NPAD], bf16, isOutput=False)
    w1_d = nc.declare_dram_parameter("w1", [P, KD1 * MH * P], bf16, isOutput=False)
    w2_d = nc.declare_dram_parameter("w2", [P, KH * MH * P], bf16, isOutput=False)
    w3_d = nc.declare_dram_parameter("w3", [P, KH * OUT], bf16, isOutput=False)
    # cstB: b1T(MH) | b2T(MH); cstLN: gamma(OUT) | beta(OUT) | b3(OUT) | eps(1)
    cstB_d = nc.declare_dram_parameter("cstB", [P, 2 * MH], f32, isOutput=False)
    cstLN_d = nc.declare_dram_parameter("cstLN", [P, 3 * OUT + 1], f32, isOutput=False)
    y_d = nc.declare_dram_parameter("y", [NPAD, OUT], bf16, isOutput=True)

    groups = []
    t0 = 0
    while t0 < T_TILES:
        g = min(GMAX, T_TILES - t0)
        groups.append((t0, g))
        t0 += g

    with tile.TileContext(nc) as tc:
        with (
            tc.tile_pool(name="const", bufs=1) as constp,
            tc.tile_pool(name="eft", bufs=6) as eftp,
            tc.tile_pool(name="agg", bufs=3) as aggp,
            tc.tile_pool(name="nfx", bufs=2) as nfxp,
            tc.tile_pool(name="h1", bufs=2) as h1p,
            tc.tile_pool(name="h2", bufs=2) as h2p,
            tc.tile_pool(name="yo", bufs=3) as yop,
            tc.tile_pool(name="st", bufs=8) as stp,
            tc.tile_pool(name="psM", bufs=4, space="PSUM") as psM,
            tc.tile_pool(name="psY", bufs=3, space="PSUM") as psY,
        ):
            cstB_sb = constp.tile([P, 2 * MH], f32)
            nc.sync.dma_start(out=cstB_sb[:], in_=cstB_d[:, :])
            w1_sb = constp.tile([P, MH * KD1 * P], bf16)
            w2_sb = constp.tile([P, MH * KH * P], bf16)
            w3_sb = constp.tile([P, KH * OUT], bf16)
            cstLN_sb = constp.tile([P, 3 * OUT + 1], f32)

            nfT_ap = nfT_d[:, :].rearrange("(k p) n -> p k n", p=P)

            def aggT_half(aggT, s, hi):
                h = P // 2
                return aggT[:, s * P + hi * h:s * P + (hi + 1) * h]

            def emit_reduce(t, aggT, s):
                """Segment-sum of one 128-node tile into aggT[:, s*P:(s+1)*P].

                eft holds each node's incoming edge features padded to dp[t]
                slots; a single DVE add-reduce over the slot axis yields the
                [96 feat, 128 node] aggregate (fp32 internally, bf16 out)."""
                cols = P * dp[t]
                if cols == 0:
                    nc.vector.memset(aggT[:, s * P:(s + 1) * P], 0.0)
                    return
                eft_t = eftp.tile([EDGE_DIM, P * dpmax], bf16, tag="eft",
                                  name="eft_t")
                nc.sync.dma_start(out=eft_t[:, 0:cols],
                                  in_=eft_d[:, off[t]:off[t] + cols])
                with nc.allow_low_precision(
                        reason="DVE reduce accumulates fp32 internally; bf16 "
                               "output matches the baseline agg precision"):
                    nc.vector.tensor_reduce(
                        out=aggT[:, s * P:(s + 1) * P],
                        in_=eft_t[:, 0:cols].rearrange("p (n d) -> p n d",
                                                       d=dp[t]),
                        axis=Axis.X,
                        op=Alu.add,
                    )

            # group 0's aggregation up front (weight DMAs stream in behind
            # it), split into half-tile DMA+reduce pairs so the DVE starts
            # summing as soon as each half lands instead of whole tiles
            agg_tiles = {}
            agg_tiles[0] = aggp.tile([EDGE_DIM, GMAX * P], bf16, tag="agg",
                                     name="aggT")
            for s in range(groups[0][1]):
                t = groups[0][0] + s
                half = (P // 2) * dp[t]
                eft_t = eftp.tile([EDGE_DIM, P * dpmax], bf16, tag="eft",
                                  name="eft_t")
                for hi in range(2):
                    nc.sync.dma_start(
                        out=eft_t[:, hi * half:(hi + 1) * half],
                        in_=eft_d[:, off[t] + hi * half:off[t] + (hi + 1) * half])
                    with nc.allow_low_precision(
                            reason="DVE reduce accumulates fp32 internally"):
                        nc.vector.tensor_reduce(
                            out=aggT_half(agg_tiles[0], s, hi),
                            in_=eft_t[:, hi * half:(hi + 1) * half].rearrange(
                                "p (n d) -> p n d", d=dp[t]),
                            axis=Axis.X,
                            op=Alu.add,
                        )

            for gi, (tstart, g) in enumerate(groups):
                nt = g * P  # free-dim width of this super-tile
                n0 = tstart * P
                aggT = agg_tiles.pop(gi)

                # ---- node features (pre-transposed on host) ----
                nfx = nfxp.tile([P, KD, GMAX * P], bf16, tag="nfx")
                nc.sync.dma_start(out=nfx[:, :, 0:nt], in_=nfT_ap[:, :, n0:n0 + nt])
                if gi == 0:
                    # per-m weight slices stream in behind group 0's agg work so
                    # layer 1/2 can begin as soon as their own slice lands
                    for m in range(MH):
                        nc.sync.dma_start(
                            out=w1_sb[:, m * KD1 * P:(m + 1) * KD1 * P],
                            in_=w1_d[:, m * KD1 * P:(m + 1) * KD1 * P])

                # ---- layer 1: h1T[m] = relu(W1.T @ xT + b1), x = [nf; agg] ----
                h1 = h1p.tile([P, KH, GMAX * P], bf16, tag="h1")
                for m in range(MH):
                    ps = psM.tile([P, GMAX * P], f32, tag="psM")
                    for k in range(KD):
                        nc.tensor.matmul(
                            out=ps[:, 0:nt],
                            lhsT=w1_sb[:, (m * KD1 + k) * P:(m * KD1 + k + 1) * P],
                            rhs=nfx[:, k, 0:nt],
                            start=(k == 0),
                            stop=False,
                        )
                    nc.tensor.matmul(
                        out=ps[:, 0:nt],
                        lhsT=w1_sb[0:EDGE_DIM, (m * KD1 + KD) * P:(m * KD1 + KD) * P + P],
                        rhs=aggT[:, 0:nt],
                        start=False,
                        stop=True,
                    )
                    nc.scalar.activation(
                        out=h1[:, m, 0:nt], in_=ps[:, 0:nt], func=Act.Relu,
                        bias=cstB_sb[:, m:m + 1],
                    )
                    if gi == 0:
                        nc.sync.dma_start(
                            out=w2_sb[:, m * KH * P:(m + 1) * KH * P],
                            in_=w2_d[:, m * KH * P:(m + 1) * KH * P])

                # ---- layer 2 ----
                h2 = h2p.tile([P, KH, GMAX * P], bf16, tag="h2")
                for m in range(MH):
                    ps = psM.tile([P, GMAX * P], f32, tag="psM")
                    for k in range(KH):
                        nc.tensor.matmul(
                            out=ps[:, 0:nt],
                            lhsT=w2_sb[:, (m * KH + k) * P:(m * KH + k + 1) * P],
                            rhs=h1[:, k, 0:nt],
                            start=(k == 0),
                            stop=(k == KH - 1),
                        )
                    nc.scalar.activation(
                        out=h2[:, m, 0:nt], in_=ps[:, 0:nt], func=Act.Relu,
                        bias=cstB_sb[:, MH + m:MH + m + 1],
                    )
                    if gi == 0 and m < 2:
                        if m == 0:
                            nc.sync.dma_start(out=w3_sb[:], in_=w3_d[:, :])
                        else:
                            nc.sync.dma_start(out=cstLN_sb[:], in_=cstLN_d[:, :])

                # ---- layer 3 (nodes on partitions) + LayerNorm ----
                # the next group's eft DMAs + DVE reduces interleave here so
                # its aggT is ready before layer 1 of group gi+1 needs it.
                if gi + 1 < len(groups):
                    tstart_nx, g_nx = groups[gi + 1]
                    agg_tiles[gi + 1] = aggp.tile([EDGE_DIM, GMAX * P], bf16,
                                                  tag="agg", name="aggT")
                else:
                    tstart_nx, g_nx = 0, 0
                for s in range(max(g, g_nx)):
                    if s < g_nx:
                        emit_reduce(tstart_nx + s, agg_tiles[gi + 1], s)
                    if s >= g:
                        continue
                    ps_y = psY.tile([P, OUT], f32, tag="psY")
                    for k in range(KH):
                        nc.tensor.matmul(
                            out=ps_y[:],
                            lhsT=h2[:, k, s * P:(s + 1) * P],
                            rhs=w3_sb[:, k * OUT:(k + 1) * OUT],
                            start=(k == 0),
                            stop=(k == KH - 1),
                        )
                    # + b3 (broadcast rows) on VectorE, off the TensorE critical path
                    nc.vector.tensor_tensor(
                        out=ps_y[:], in0=ps_y[:],
                        in1=cstLN_sb[:, 2 * OUT:3 * OUT],
                        op=Alu.add,
                    )
                    st6 = stp.tile([P, 6], f32, tag="st6")
                    nc.vector.bn_stats(st6[:], ps_y[:])
                    mv = stp.tile([P, 2], f32, tag="mv")
                    nc.vector.bn_aggr(mv[:], st6[:])
                    std = stp.tile([P, 1], f32, tag="std")
                    nc.scalar.activation(std[:], mv[:, 1:2], Act.Sqrt,
                                         bias=cstLN_sb[:, 3 * OUT:])
                    rstd = stp.tile([P, 1], f32, tag="rstd")
                    nc.vector.reciprocal(rstd[:], std[:])
                    nmr = stp.tile([P, 1], f32, tag="nmr")
                    nc.vector.tensor_scalar(
                        out=nmr[:], in0=mv[:, 0:1], scalar1=rstd[:], scalar2=-1.0,
                        op0=Alu.mult, op1=Alu.mult,
                    )
                    yn = yop.tile([P, OUT], bf16, tag="yn")
                    nc.scalar.activation(
                        out=yn[:], in_=ps_y[:], func=Act.Identity,
                        bias=nmr[:], scale=rstd[:],
                    )
                    if apply_gamma_beta:
                        nc.vector.tensor_tensor(
                            out=yn[:], in0=yn[:],
                            in1=cstLN_sb[:, 0:OUT], op=Alu.mult,
                        )
                        nc.vector.tensor_tensor(
                            out=yn[:], in0=yn[:],
                            in1=cstLN_sb[:, OUT:2 * OUT], op=Alu.add,
                        )
                    r0 = (tstart + s) * P
                    nc.sync.dma_start(out=y_d[r0:r0 + P, :], in_=yn[:])

    nc.compile()
    return nc


# ----------------------------------------------------------------------------
# Host-side sharding / layout prep
# ----------------------------------------------------------------------------

def _prep_core(c, node_feat, edge_feat, dst, order, dp, off, total_cols):
    """Degree-sorted node layout for one core.

    order: ascending-degree permutation of this core's nodes; sorted node i
    sits at padded position NSHIFT + i (tile = pos >> 7, rel = pos & 127).
    eft[f, off[t] + rel*dp[t] + slot] = edge_feat[e, f] for the slot-th
    incoming edge of that node; unused slots stay zero.
    """
    lo = c * NPC
    pos_of_node = np.empty(NPC, np.int64)
    pos_of_node[order] = NSHIFT + np.arange(NPC)

    sel = np.flatnonzero((dst >= lo) & (dst < lo + NPC))
    p = pos_of_node[dst[sel] - lo]
    eorder = np.argsort(p, kind="stable")
    sel = sel[eorder]
    p = p[eorder]
    # slot index within each node's run
    node_starts = np.zeros(NPAD, np.int64)
    cnts = np.bincount(p, minlength=NPAD)
    np.cumsum(cnts[:-1], out=node_starts[1:])
    slot = np.arange(p.size) - node_starts[p]
    t = p >> 7
    rel = p & 127
    cols = off[t] + rel * dp[t] + slot
    assert slot.max(initial=0) < dp[t].max(initial=1) and cols.max(initial=0) < total_cols

    eft = np.zeros((EDGE_DIM, total_cols), BF16)
    eft[:, cols] = edge_feat[sel].astype(BF16).T

    nfT = np.zeros((NODE_DIM, NPAD), BF16)
    nfT[:, NSHIFT:] = node_feat[lo + order].T.astype(BF16)
    return {"eft": eft, "nfT": nfT}


def _prep_shared(W1, b1, W2, b2, W3, b3, gamma, beta):
    KD1 = NODE_DIM // P + 1
    MH = HID // P
    KH = HID // P

    w1p = np.zeros((KD1 * P, HID), np.float32)
    w1p[:NODE_DIM + EDGE_DIM] = W1
    # m-major: col index (m*KD1 + k)*P + j
    w1 = np.ascontiguousarray(
        w1p.reshape(KD1, P, MH, P).transpose(1, 2, 0, 3)).reshape(P, -1).astype(BF16)
    w2 = np.ascontiguousarray(
        W2.reshape(KH, P, MH, P).transpose(1, 2, 0, 3)).reshape(P, -1).astype(BF16)
    w3 = np.ascontiguousarray(
        W3.reshape(KH, P, OUT).transpose(1, 0, 2)).reshape(P, -1).astype(BF16)

    cstB = np.ascontiguousarray(np.concatenate(
        [b1.reshape(MH, P).T, b2.reshape(MH, P).T], axis=1).astype(np.float32))
    cstLN = np.ascontiguousarray(np.concatenate([
        np.tile(gamma.reshape(1, OUT), (P, 1)),
        np.tile(beta.reshape(1, OUT), (P, 1)),
        np.tile(b3.reshape(1, OUT), (P, 1)),
        np.full((P, 1), LN_EPS, np.float32),
    ], axis=1).astype(np.float32))
    return {"w1": w1, "w2": w2, "w3": w3, "cstB": cstB, "cstLN": cstLN}


# ----------------------------------------------------------------------------
# Entry point
# ----------------------------------------------------------------------------

def _ensure_axon_hooks_importable():
    """bass_utils imports antenv.axon_hooks when tracing is requested (even via
    the BASS_TRACE env var); provide a no-op stub if the module is absent so
    that path degrades to trace-skipped instead of crashing."""
    try:
        import antenv.axon_hooks  # noqa: F401
    except Exception:
        import sys
        import types
        try:
            import antenv
        except Exception:
            return
        mod = types.ModuleType('antenv.axon_hooks')
        mod._hook = None
        mod.set_axon_ntff_profile_hook = lambda h: setattr(mod, '_hook', h)
        mod.get_axon_ntff_profile_hook = lambda: mod._hook
        sys.modules['antenv.axon_hooks'] = mod
        antenv.axon_hooks = mod


def kernel(node_feat, edge_feat, edge_index, n_nodes, W1, b1, W2, b2, W3, b3,
           gamma, beta, _want_trace=False):
    from concourse.bass_utils import run_bass_kernel_spmd
    _ensure_axon_hooks_importable()

    node_feat = np.asarray(node_feat, dtype=np.float32)
    edge_feat = np.asarray(edge_feat, dtype=np.float32)
    edge_index = np.asarray(edge_index)
    assert int(n_nodes) == N_NODES
    assert node_feat.shape == (N_NODES, NODE_DIM)
    assert edge_feat.shape == (N_EDGES, EDGE_DIM)

    dst = edge_index[1].astype(np.int64)

    # degree-sorted node order per core; global per-tile slot counts dp[t]
    counts = np.bincount(dst, minlength=N_NODES)
    orders = []
    dp = np.zeros(T_TILES, np.int64)
    for c in range(NCORES):
        degc = counts[c * NPC:(c + 1) * NPC]
        order = np.argsort(degc, kind="stable")
        orders.append(order)
        padded = np.zeros(NPAD, np.int64)
        padded[NSHIFT:] = degc[order]
        dp = np.maximum(dp, padded.reshape(T_TILES, P).max(axis=1))
    off = np.zeros(T_TILES, np.int64)
    np.cumsum(P * dp[:-1], out=off[1:])
    total_cols = int(off[-1] + P * dp[-1])

    gamma = np.asarray(gamma, dtype=np.float32)
    beta = np.asarray(beta, dtype=np.float32)
    apply_gb = not (np.all(gamma == 1.0) and np.all(beta == 0.0))

    key = (tuple(int(x) for x in dp), apply_gb)
    if key not in _CACHE:
        _CACHE[key] = _build_program(key[0], apply_gb)
    nc = _CACHE[key]

    shared = _prep_shared(
        np.asarray(W1, np.float32), np.asarray(b1, np.float32),
        np.asarray(W2, np.float32), np.asarray(b2, np.float32),
        np.asarray(W3, np.float32), np.asarray(b3, np.float32),
        gamma, beta)

    in_maps = []
    for c in range(NCORES):
        m = _prep_core(c, node_feat, edge_feat, dst, orders[c], dp, off,
                       total_cols)
        m.update(shared)
        in_maps.append(m)

    res = run_bass_kernel_spmd(nc, in_maps, list(range(NCORES)), trace=_want_trace)

    out = np.empty((N_NODES, OUT), np.float32)
    for c in range(NCORES):
        yc = np.asarray(res.results[c]["y"][NSHIFT:], dtype=np.float32)
        out[c * NPC + orders[c]] = yc
    if _want_trace:
        kernel.last_results = res
    return out


kernel.last_results = None
